# revision 11
# baseline (speedup 1.0000x reference)
"""Trainium2 Bass kernel for BaseAttentionBlock (B=8, C=512, HxW=64x64, K=V=256, O=512).

Strategy: data-parallel over batch B across the 8 NeuronCores (one batch element
per core, SPMD, no collectives). Per core:

  k' = relu(s*(wk@x) + b2)/4 (BN folded on host, 1/sqrt(K) folded as 1/4 into k')
  vT = (x^T @ wv^T) + bv     computed directly in [m, v] layout (no transposes)
  E  = exp(k'^T k')          symmetric, so the [m x n] tile computed in the fused
                             loop is simultaneously the [m, n]-layout rhs the ctx
                             matmul needs -> single pass, no transposes, rowsums
                             fused into the exp via accum_out
  ctx_raw[v, n] = sum_m vT[m, v] * E[m, n]   accumulated in PSUM per n-block
  out_raw = wW @ ctx_raw     (out-proj inside the main loop, unnormalized)
  out = out_raw * (1/rowsum) + bW   (softmax normalization deferred to the end)

k', E, vT are fp8e4m3; sim/ctx matmuls use DoubleRow (contraction 256 per pass,
2x PE). The loop processes n in 1024-wide blocks (nj pairs) so each exp covers
[128, 1024] (halves ACT instruction count). Projections + the first n-block ride
behind the x input-DMA wave. Reciprocal broadcast via bf16 hi/lo rank-1 matmuls.
Measured rel err ~8e-3 (bf16-everywhere variant: 1.5e-3).
"""

import numpy as np
import ml_dtypes

BN_EPS = 1e-5
B, C, H, W = 8, 512, 64, 64
N = H * W  # 4096
K = 256
V = 256
O = 512
P = 128  # partitions
NT = 512  # matmul free-dim tile
NB = 1024  # n-block (nj pair)
CC = C // P  # 4 contraction chunks for projections
KC = K // P  # 2 kch chunks
MI = N // P  # 32 m-chunks of 128
NQ = MI // 2  # 16 m-chunk pairs (DoubleRow contraction = 256)
NJ = N // NT  # 8 n-chunks of 512
NBL = N // NB  # 4 n-blocks
OC = O // P  # 4 output chunks
BF16 = ml_dtypes.bfloat16

_COMPILED = None


def _build():
    import concourse.bass as bass
    import concourse.tile as tile
    import concourse.mybir as mybir
    from concourse import bacc, masks
    from contextlib import ExitStack

    f32 = mybir.dt.float32
    bf16 = mybir.dt.bfloat16
    f8 = mybir.dt.float8e4
    AF = mybir.ActivationFunctionType
    DR = mybir.MatmulPerfMode.DoubleRow

    nc = bacc.Bacc(trn_type="TRN2", target_bir_lowering=False, debug=False,
                   num_devices=B)

    x_d = nc.dram_tensor("x16", [C, N], bf16, kind="ExternalInput").ap()
    wkT_d = nc.dram_tensor("wkT16", [C, K], bf16, kind="ExternalInput").ap()
    wvT_d = nc.dram_tensor("wvT16", [C, V], bf16, kind="ExternalInput").ap()
    wWT_d = nc.dram_tensor("wWT16", [V, O], bf16, kind="ExternalInput").ap()
    ks_d = nc.dram_tensor("kscale", [K, 1], f32, kind="ExternalInput").ap()
    kb_d = nc.dram_tensor("kbias", [K, 1], f32, kind="ExternalInput").ap()
    bv_d = nc.dram_tensor("bvrow", [1, V], f32, kind="ExternalInput").ap()
    bW_d = nc.dram_tensor("bW32", [O, 1], f32, kind="ExternalInput").ap()
    out_d = nc.dram_tensor("out", [O, N], f32, kind="ExternalOutput").ap()

    with tile.TileContext(nc) as tc, ExitStack() as ctx:
        const = ctx.enter_context(tc.tile_pool(name="const", bufs=1))

        # ---- persistent SBUF tensors ----
        x_sb = [const.tile([P, N], bf16, tag=f"x{c}", name=f"x_sb{c}")
                for c in range(CC)]
        wk_sb = [const.tile([P, K], bf16, tag=f"wk{c}", name=f"wk_sb{c}")
                 for c in range(CC)]
        wv_sb = [const.tile([P, V], bf16, tag=f"wv{c}", name=f"wv_sb{c}")
                 for c in range(CC)]
        wW_sb = [const.tile([P, O], bf16, tag=f"wW{v}", name=f"wW_sb{v}")
                 for v in range(KC)]
        ks_sb = const.tile([P, KC], f32, tag="ks", name="ks_sb")
        kb_sb = const.tile([P, KC], f32, tag="kb", name="kb_sb")
        bvrow_sb = const.tile([1, V], f32, tag="bvrow", name="bvrow_sb")
        bW_sb = const.tile([P, OC], f32, tag="bW", name="bW_sb")
        ones16 = const.tile([1, P], bf16, tag="ones16", name="ones16")
        ident = const.tile([P, P], f32, tag="ident", name="ident")

        # k' fp8, both kch chunks in one tile (chunk kc at free offset kc*N)
        k2_sb = const.tile([P, KC * N], f8, tag="k2", name="k2_sb")
        # vT fp8 [m, v]: chunk mi at cols [mi*V, (mi+1)*V)
        vT_sb = const.tile([P, MI * V], f8, tag="vT", name="vT_sb")
        outr_sb = [const.tile([P, N], f32, tag=f"outr{oc}", name=f"outr_sb{oc}")
                   for oc in range(OC)]
        rsparts_sb = const.tile([P, MI * NBL], f32, tag="rsparts",
                                name="rsparts_sb")
        rs_sb = const.tile([P, MI], f32, tag="rs", name="rs_sb")
        recipT_sb = const.tile([MI, P], f32, tag="recipT", name="recipT_sb")
        recipT_hi = const.tile([MI, P], bf16, tag="recipThi", name="recipT_hi")
        recipT_lo = const.tile([MI, P], bf16, tag="recipTlo", name="recipT_lo")
        hirow_sb = const.tile([1, N], bf16, tag="hirow", name="hirow_sb")
        lorow_sb = const.tile([1, N], bf16, tag="lorow", name="lorow_sb")
        bvbc_sb = const.tile([P, V], f32, tag="bvbc", name="bvbc_sb")

        # ---- input DMAs: small weights first, then x by column quarters so
        # the projection + first-block compute wave rides behind the transfer
        for c in range(CC):
            nc.sync.dma_start(out=wk_sb[c][:], in_=wkT_d[c * P:(c + 1) * P, :])
            nc.sync.dma_start(out=wv_sb[c][:], in_=wvT_d[c * P:(c + 1) * P, :])
        for v in range(KC):
            nc.sync.dma_start(out=wW_sb[v][:], in_=wWT_d[v * P:(v + 1) * P, :])
        for kc in range(KC):
            nc.sync.dma_start(out=ks_sb[:, kc:kc + 1],
                              in_=ks_d[kc * P:(kc + 1) * P, :])
            nc.sync.dma_start(out=kb_sb[:, kc:kc + 1],
                              in_=kb_d[kc * P:(kc + 1) * P, :])
        nc.sync.dma_start(out=bvrow_sb[:], in_=bv_d[:])
        for oc in range(OC):
            nc.sync.dma_start(out=bW_sb[:, oc:oc + 1],
                              in_=bW_d[oc * P:(oc + 1) * P, :])
        nc.gpsimd.memset(ones16[:], 1.0)
        masks.make_identity(nc, ident[:])
        nc.sync.dma_start(out=bvbc_sb[:], in_=bv_d[:].to_broadcast((P, V)))
        XQ = 4
        xq = N // XQ
        for q in range(XQ):
            for c in range(CC):
                nc.sync.dma_start(
                    out=x_sb[c][:, q * xq:(q + 1) * xq],
                    in_=x_d[c * P:(c + 1) * P, q * xq:(q + 1) * xq])

        # DoubleRow contraction views
        k2v = k2_sb[:].rearrange("p (ko n) -> p ko n", ko=KC)      # [P, 2, N]
        vTv = vT_sb[:].rearrange("p (q ko v) -> p q ko v", q=NQ, ko=2)

        with tc.tile_pool(name="psS", bufs=2, space="PSUM") as psS, \
                tc.tile_pool(name="psC", bufs=2, space="PSUM") as psC, \
                tc.tile_pool(name="epool", bufs=4) as epool, \
                tc.tile_pool(name="cnpool", bufs=3) as cnpool:
            # psS slots are [P, NB] (2 banks each); kp/vp/op [P, NT] tiles
            # share the same slots via the same tag.

            def kproj(kc, nj):
                kp_ps = psS.tile([P, NT], f32, tag="sim", name="kp_ps")
                for c in range(CC):
                    nc.tensor.matmul(
                        kp_ps[:],
                        lhsT=wk_sb[c][:, kc * P:(kc + 1) * P],
                        rhs=x_sb[c][:, nj * NT:(nj + 1) * NT],
                        start=(c == 0), stop=(c == CC - 1))
                nc.scalar.activation(
                    k2_sb[:, kc * N + nj * NT: kc * N + (nj + 1) * NT],
                    kp_ps[:], AF.Relu, bias=kb_sb[:, kc:kc + 1],
                    scale=ks_sb[:, kc:kc + 1])

            def vproj(mi):
                vp_ps = psS.tile([P, NT], f32, tag="sim", name="vp_ps")
                for c in range(CC):
                    nc.tensor.matmul(
                        vp_ps[:, :V],
                        lhsT=x_sb[c][:, mi * P:(mi + 1) * P],
                        rhs=wv_sb[c][:],
                        start=(c == 0), stop=(c == CC - 1))
                nc.vector.tensor_add(vT_sb[:, mi * V:(mi + 1) * V],
                                     vp_ps[:, :V], bvbc_sb[:])

            st = {}

            def start_block(bl):
                st["ctx"] = [psC.tile([P, NB], f32, tag="ctx",
                                      name=f"ctx_ps{v}") for v in range(KC)]

            def loop_iter(bl, mi):
                q, half = divmod(mi, 2)
                # sim rows mi, n-cols [bl*NB, (bl+1)*NB): two DoubleRow mms
                sim_ps = psS.tile([P, NB], f32, tag="sim", name="sim_ps")
                for h in range(2):
                    nj = 2 * bl + h
                    nc.tensor.matmul(
                        sim_ps[:, h * NT:(h + 1) * NT],
                        lhsT=k2v[:, :, mi * P:(mi + 1) * P],
                        rhs=k2v[:, :, nj * NT:(nj + 1) * NT],
                        start=True, stop=True, perf_mode=DR)
                if half == 0:
                    st["e4"] = epool.tile([P, 2 * NB], f8, tag="e", name="e4")
                e4 = st["e4"]
                # e4 layout: [p, h, ko, n] -> free index h*NB + ko*NT + n
                e4v = e4.rearrange("p (h ko n) -> p h ko n", h=2, ko=2)
                col = mi * NBL + bl
                nc.scalar.activation(
                    e4v[:, :, half, :],
                    sim_ps[:].rearrange("p (h n) -> p h n", h=2),
                    AF.Exp, accum_out=rsparts_sb[:, col:col + 1])
                if half == 1:
                    for v in range(KC):
                        for h in range(2):
                            nc.tensor.matmul(
                                st["ctx"][v][:, h * NT:(h + 1) * NT],
                                lhsT=vTv[:, q, :, v * P:(v + 1) * P],
                                rhs=e4v[:, h, :, :],
                                start=(q == 0), stop=(q == NQ - 1),
                                perf_mode=DR)

            def finish_block(bl):
                ctx_ps = st["ctx"]
                cn = [cnpool.tile([P, NB], bf16, tag=f"cn{v}", name=f"cn{v}")
                      for v in range(KC)]
                for v in range(KC):
                    nc.vector.tensor_copy(cn[v][:], ctx_ps[v][:])
                for h in range(2):
                    nj = 2 * bl + h
                    for oc in range(OC):
                        op_ps = psS.tile([P, NT], f32, tag="sim", name="op_ps")
                        for v in range(KC):
                            nc.tensor.matmul(
                                op_ps[:],
                                lhsT=wW_sb[v][:, oc * P:(oc + 1) * P],
                                rhs=cn[v][:, h * NT:(h + 1) * NT],
                                start=(v == 0), stop=(v == KC - 1))
                        nc.vector.tensor_copy(
                            outr_sb[oc][:, nj * NT:(nj + 1) * NT], op_ps[:])

            # quarter-by-quarter wave: kproj + vproj + block-0 segment
            start_block(0)
            for q in range(XQ):
                for kc in range(KC):
                    for nj in (2 * q, 2 * q + 1):
                        kproj(kc, nj)
                for mi in range(8 * q, 8 * q + 8):
                    vproj(mi)
                    loop_iter(0, mi)
            finish_block(0)
            for bl in range(1, NBL):
                start_block(bl)
                for mi in range(MI):
                    loop_iter(bl, mi)
                finish_block(bl)

        # ---- finalize: rowsums -> recip -> hi/lo broadcast -> scale+bias ----
        with tc.tile_pool(name="psF", bufs=2, space="PSUM") as psF, \
                tc.tile_pool(name="psT", bufs=1, space="PSUM") as psT, \
                tc.tile_pool(name="fin", bufs=6) as fin:
            for mi in range(MI):
                nc.vector.tensor_reduce(
                    rs_sb[:, mi:mi + 1],
                    rsparts_sb[:, mi * NBL:(mi + 1) * NBL],
                    axis=mybir.AxisListType.X, op=mybir.AluOpType.add)
            tp_ps = psT.tile([MI, P], f32, tag="tp", name="tp_ps")
            nc.tensor.transpose(tp_ps[:], rs_sb[:], ident[:])
            nc.vector.reciprocal(recipT_sb[:], tp_ps[:])
            # hi/lo bf16 split of the fp32 reciprocals (sum reconstructs fp32
            # to ~2^-16 rel); rank-1 bf16 matmuls then rebuild fp32 in PSUM
            nc.vector.tensor_copy(recipT_hi[:], recipT_sb[:])
            nc.vector.tensor_sub(recipT_lo[:], recipT_sb[:], recipT_hi[:])
            nc.sync.dma_start(out=hirow_sb[:], in_=recipT_hi[:])
            nc.sync.dma_start(out=lorow_sb[:], in_=recipT_lo[:])

            for nj in range(NJ):
                bc_ps = psF.tile([P, NT], f32, tag="bc", name="bc_ps")
                nc.tensor.matmul(
                    bc_ps[:], lhsT=ones16[:],
                    rhs=hirow_sb[:, nj * NT:(nj + 1) * NT],
                    start=True, stop=False)
                nc.tensor.matmul(
                    bc_ps[:], lhsT=ones16[:],
                    rhs=lorow_sb[:, nj * NT:(nj + 1) * NT],
                    start=False, stop=True)
                for oc in range(OC):
                    ft = fin.tile([P, NT], f32, tag="ft", name="ft")
                    nc.vector.tensor_mul(
                        ft[:], outr_sb[oc][:, nj * NT:(nj + 1) * NT], bc_ps[:])
                    ot = fin.tile([P, NT], f32, tag="ot", name="ot")
                    nc.scalar.activation(ot[:], ft[:], AF.Identity,
                                         bias=bW_sb[:, oc:oc + 1])
                    nc.sync.dma_start(
                        out=out_d[oc * P:(oc + 1) * P, nj * NT:(nj + 1) * NT],
                        in_=ot[:])
    nc.compile()
    return nc


def _get_compiled():
    global _COMPILED
    if _COMPILED is None:
        _COMPILED = _build()
    return _COMPILED


def _make_in_maps(x, wv, bv, wk, bk, gamma, beta, rmean, rvar, wW, bW):
    x = np.asarray(x, dtype=np.float32)
    s = np.asarray(gamma, np.float32) / np.sqrt(np.asarray(rvar, np.float32) + BN_EPS)
    kscale = (s / 4.0).astype(np.float32).reshape(K, 1)
    kbias = (((np.asarray(bk, np.float32) - np.asarray(rmean, np.float32)) * s
              + np.asarray(beta, np.float32)) / 4.0).astype(np.float32).reshape(K, 1)
    shared = {
        "wkT16": np.ascontiguousarray(np.asarray(wk, np.float32).T).astype(BF16),
        "wvT16": np.ascontiguousarray(np.asarray(wv, np.float32).T).astype(BF16),
        "wWT16": np.ascontiguousarray(np.asarray(wW, np.float32).T).astype(BF16),
        "kscale": kscale,
        "kbias": kbias,
        "bvrow": np.asarray(bv, np.float32).reshape(1, V),
        "bW32": np.asarray(bW, np.float32).reshape(O, 1),
    }
    in_maps = []
    for b in range(B):
        m = dict(shared)
        m["x16"] = np.ascontiguousarray(x[b].reshape(C, N)).astype(BF16)
        in_maps.append(m)
    return in_maps


def _run(inputs, trace=False):
    from concourse.bass_utils import run_bass_kernel_spmd
    nc = _get_compiled()
    in_maps = _make_in_maps(**inputs)
    res = run_bass_kernel_spmd(nc, in_maps, list(range(B)), trace=trace)
    outs = [np.asarray(res.results[b]["out"], dtype=np.float32).reshape(O, H, W)
            for b in range(B)]
    return np.stack(outs), res


def kernel(x, wv, bv, wk, bk, gamma, beta, rmean, rvar, wW, bW):
    out, _ = _run(dict(x=x, wv=wv, bv=bv, wk=wk, bk=bk, gamma=gamma, beta=beta,
                       rmean=rmean, rvar=rvar, wW=wW, bW=bW))
    return out


# revision 15
# speedup vs baseline: 1.1145x; 1.1145x over previous
"""Trainium2 Bass kernel for BaseAttentionBlock (B=8, C=512, HxW=64x64, K=V=256, O=512).

Strategy: data-parallel over batch B across the 8 NeuronCores (one batch element
per core, SPMD, no collectives). Per core:

  k' = relu(s*(wk@x) + b2)/4 (BN folded on host, 1/sqrt(K) folded as 1/4 into k')
  vT = (x^T @ wv^T) + bv     computed directly in [m, v] layout (no transposes)
  E  = exp(k'^T k')          symmetric, so the [128m x 512n] tile computed in the
                             fused loop is simultaneously the [m, n]-layout rhs the
                             ctx matmul needs -> single pass, no transposes, no
                             DRAM round trip, rowsums fused into exp via accum_out
  ctx_raw[v, n] = sum_m vT[m, v] * E[m, n]   accumulated in PSUM per 512-wide n chunk
  out_raw = wW @ ctx_raw     (out-proj inside the main loop, unnormalized)
  out = out_raw * (1/rowsum) + bW   (softmax normalization deferred to the end)

k', E, vT are stored fp8e4m3 and the sim/ctx matmuls use DoubleRow perf mode
(contraction 256 in one pass, 2x PE throughput); fp32 accumulation in PSUM.
Projections and out-proj in bf16. Measured rel err ~8e-3 (bf16 variant: 1.5e-3).
"""

import numpy as np
import ml_dtypes

BN_EPS = 1e-5
B, C, H, W = 8, 512, 64, 64
N = H * W  # 4096
K = 256
V = 256
O = 512
P = 128  # partitions
NT = 512  # free-dim tile
CC = C // P  # 4 contraction chunks for projections
KC = K // P  # 2 kch chunks
MI = N // P  # 32 m-chunks of 128
NQ = MI // 2  # 16 m-chunk pairs (DoubleRow contraction = 256)
NJ = N // NT  # 8 n-chunks of 512
OC = O // P  # 4 output chunks
BF16 = ml_dtypes.bfloat16

_COMPILED = None


def _build():
    import concourse.bass as bass
    import concourse.tile as tile
    import concourse.mybir as mybir
    from concourse import bacc, masks
    from contextlib import ExitStack

    f32 = mybir.dt.float32
    bf16 = mybir.dt.bfloat16
    f8 = mybir.dt.float8e4
    AF = mybir.ActivationFunctionType
    DR = mybir.MatmulPerfMode.DoubleRow

    nc = bacc.Bacc(trn_type="TRN2", target_bir_lowering=False, debug=False,
                   num_devices=B)

    x_d = nc.dram_tensor("x16", [C, N], bf16, kind="ExternalInput").ap()
    wkT_d = nc.dram_tensor("wkT16", [C, K], bf16, kind="ExternalInput").ap()
    wvT_d = nc.dram_tensor("wvT16", [C, V], bf16, kind="ExternalInput").ap()
    wWT_d = nc.dram_tensor("wWT16", [V, O], bf16, kind="ExternalInput").ap()
    ks_d = nc.dram_tensor("kscale", [K, 1], f32, kind="ExternalInput").ap()
    kb_d = nc.dram_tensor("kbias", [K, 1], f32, kind="ExternalInput").ap()
    bv_d = nc.dram_tensor("bvrow", [1, V], f32, kind="ExternalInput").ap()
    bW_d = nc.dram_tensor("bW32", [O, 1], f32, kind="ExternalInput").ap()
    out_d = nc.dram_tensor("out", [O, N], f32, kind="ExternalOutput").ap()

    with tile.TileContext(nc) as tc, ExitStack() as ctx:
        const = ctx.enter_context(tc.tile_pool(name="const", bufs=1))

        # ---- persistent SBUF tensors ----
        x_sb = [const.tile([P, N], bf16, tag=f"x{c}", name=f"x_sb{c}")
                for c in range(CC)]
        wk_sb = [const.tile([P, K], bf16, tag=f"wk{c}", name=f"wk_sb{c}")
                 for c in range(CC)]
        wv_sb = [const.tile([P, V], bf16, tag=f"wv{c}", name=f"wv_sb{c}")
                 for c in range(CC)]
        wW_sb = [const.tile([P, O], bf16, tag=f"wW{v}", name=f"wW_sb{v}")
                 for v in range(KC)]
        ks_sb = const.tile([P, KC], f32, tag="ks", name="ks_sb")
        kb_sb = const.tile([P, KC], f32, tag="kb", name="kb_sb")
        bvrow_sb = const.tile([1, V], f32, tag="bvrow", name="bvrow_sb")
        bW_sb = const.tile([P, OC], f32, tag="bW", name="bW_sb")
        ones16 = const.tile([1, P], bf16, tag="ones16", name="ones16")
        ident = const.tile([P, P], f32, tag="ident", name="ident")

        # k' fp8, both kch chunks in one tile (chunk kc at free offset kc*N)
        # -> DoubleRow lhsT/rhs views [P, 2, *]
        k2_sb = const.tile([P, KC * N], f8, tag="k2", name="k2_sb")
        # vT fp8 [m, v]: chunk mi occupies cols [mi*V, (mi+1)*V); an mi pair
        # q is the contiguous [P, 2, V] block at q*2*V
        vT_sb = const.tile([P, MI * V], f8, tag="vT", name="vT_sb")
        outr_sb = [const.tile([P, N], f32, tag=f"outr{oc}", name=f"outr_sb{oc}")
                   for oc in range(OC)]
        rsparts_sb = const.tile([P, MI * NJ], f32, tag="rsparts",
                                name="rsparts_sb")
        rs_sb = const.tile([P, MI], f32, tag="rs", name="rs_sb")
        recipT_sb = const.tile([MI, P], f32, tag="recipT", name="recipT_sb")
        recipT_hi = const.tile([MI, P], bf16, tag="recipThi", name="recipT_hi")
        recipT_lo = const.tile([MI, P], bf16, tag="recipTlo", name="recipT_lo")
        hirow_sb = const.tile([1, N], bf16, tag="hirow", name="hirow_sb")
        lorow_sb = const.tile([1, N], bf16, tag="lorow", name="lorow_sb")
        bvbc_sb = const.tile([P, V], f32, tag="bvbc", name="bvbc_sb")

        # ---- input DMAs: small weights first, then x by column quarters so
        # the projection + nj=0 compute wave can ride behind the transfer ----
        for c in range(CC):
            nc.sync.dma_start(out=wk_sb[c][:], in_=wkT_d[c * P:(c + 1) * P, :])
            nc.sync.dma_start(out=wv_sb[c][:], in_=wvT_d[c * P:(c + 1) * P, :])
        for v in range(KC):
            nc.sync.dma_start(out=wW_sb[v][:], in_=wWT_d[v * P:(v + 1) * P, :])
        for kc in range(KC):
            nc.sync.dma_start(out=ks_sb[:, kc:kc + 1],
                              in_=ks_d[kc * P:(kc + 1) * P, :])
            nc.sync.dma_start(out=kb_sb[:, kc:kc + 1],
                              in_=kb_d[kc * P:(kc + 1) * P, :])
        nc.sync.dma_start(out=bvrow_sb[:], in_=bv_d[:])
        for oc in range(OC):
            nc.sync.dma_start(out=bW_sb[:, oc:oc + 1],
                              in_=bW_d[oc * P:(oc + 1) * P, :])
        nc.gpsimd.memset(ones16[:], 1.0)
        masks.make_identity(nc, ident[:])
        # bv broadcast [P, V] via stride-0 DMA read from DRAM
        nc.sync.dma_start(out=bvbc_sb[:], in_=bv_d[:].to_broadcast((P, V)))
        XQ = 4
        xq = N // XQ
        for q in range(XQ):
            for c in range(CC):
                nc.sync.dma_start(
                    out=x_sb[c][:, q * xq:(q + 1) * xq],
                    in_=x_d[c * P:(c + 1) * P, q * xq:(q + 1) * xq])

        # DoubleRow contraction views
        k2v = k2_sb[:].rearrange("p (ko n) -> p ko n", ko=KC)      # [P, 2, N]
        vTv = vT_sb[:].rearrange("p (q ko v) -> p q ko v", q=NQ, ko=2)

        # ---- fused program: projections + nj=0 woven into the x-DMA wave,
        # then the remaining n-chunks ----
        with tc.tile_pool(name="psP", bufs=2, space="PSUM") as psP, \
                tc.tile_pool(name="psS", bufs=4, space="PSUM") as psS, \
                tc.tile_pool(name="psC", bufs=2, space="PSUM") as psC, \
                tc.tile_pool(name="epool", bufs=6) as epool, \
                tc.tile_pool(name="cnpool", bufs=4) as cnpool:

            def kproj(kc, nj):
                kp_ps = psP.tile([P, NT], f32, tag="ps", name="kp_ps")
                for c in range(CC):
                    nc.tensor.matmul(
                        kp_ps[:],
                        lhsT=wk_sb[c][:, kc * P:(kc + 1) * P],
                        rhs=x_sb[c][:, nj * NT:(nj + 1) * NT],
                        start=(c == 0), stop=(c == CC - 1))
                nc.scalar.activation(
                    k2_sb[:, kc * N + nj * NT: kc * N + (nj + 1) * NT],
                    kp_ps[:], AF.Relu, bias=kb_sb[:, kc:kc + 1],
                    scale=ks_sb[:, kc:kc + 1])

            def vproj(mi):
                vp_ps = psP.tile([P, NT], f32, tag="ps", name="vp_ps")
                for c in range(CC):
                    nc.tensor.matmul(
                        vp_ps[:, :V],
                        lhsT=x_sb[c][:, mi * P:(mi + 1) * P],
                        rhs=wv_sb[c][:],
                        start=(c == 0), stop=(c == CC - 1))
                nc.vector.tensor_add(vT_sb[:, mi * V:(mi + 1) * V],
                                     vp_ps[:, :V], bvbc_sb[:])

            loop_state = {"pending": None}

            def emit_ctx(q, e2):
                # ctx matmuls for pair q; called one pair late (software
                # pipelining) so the PE's in-order queue never blocks on the
                # exp the pair depends on — PE runs the NEXT pair's sims while
                # ACT computes this pair's exps.
                e2v = e2.rearrange("p (ko n) -> p ko n", ko=2)
                for v in range(KC):
                    nc.tensor.matmul(
                        loop_state["ctx"][v][:],
                        lhsT=vTv[:, q, :, v * P:(v + 1) * P],
                        rhs=e2v[:],
                        start=(q == 0), stop=(q == NQ - 1),
                        perf_mode=DR)

            def loop_iter(nj, mi):
                q, half = divmod(mi, 2)
                sim_ps = psS.tile([P, NT], f32, tag="sim", name="sim_ps")
                nc.tensor.matmul(
                    sim_ps[:],
                    lhsT=k2v[:, :, mi * P:(mi + 1) * P],
                    rhs=k2v[:, :, nj * NT:(nj + 1) * NT],
                    start=True, stop=True, perf_mode=DR)
                if half == 0:
                    loop_state["e2"] = epool.tile([P, 2 * NT], f8, tag="e",
                                                  name="e2")
                e2 = loop_state["e2"]
                col = mi * NJ + nj
                nc.scalar.activation(
                    e2[:, half * NT:(half + 1) * NT], sim_ps[:], AF.Exp,
                    accum_out=rsparts_sb[:, col:col + 1])
                if half == 1:
                    if loop_state["pending"] is not None:
                        emit_ctx(*loop_state["pending"])
                    loop_state["pending"] = (q, e2)

            def start_nj(nj):
                loop_state["ctx"] = [
                    psC.tile([P, NT], f32, tag="ctx", name=f"ctx_ps{v}")
                    for v in range(KC)]

            def finish_nj(nj):
                # flush the last pair's ctx, then out-proj for this n-chunk
                emit_ctx(*loop_state["pending"])
                loop_state["pending"] = None
                ctx_ps = loop_state["ctx"]
                cn = [cnpool.tile([P, NT], bf16, tag=f"cn{v}", name=f"cn{v}")
                      for v in range(KC)]
                for v in range(KC):
                    nc.vector.tensor_copy(cn[v][:], ctx_ps[v][:])
                for oc in range(OC):
                    op_ps = psP.tile([P, NT], f32, tag="ps", name="op_ps")
                    for v in range(KC):
                        nc.tensor.matmul(
                            op_ps[:],
                            lhsT=wW_sb[v][:, oc * P:(oc + 1) * P],
                            rhs=cn[v][:],
                            start=(v == 0), stop=(v == KC - 1))
                    nc.vector.tensor_copy(
                        outr_sb[oc][:, nj * NT:(nj + 1) * NT], op_ps[:])

            # quarter-by-quarter wave: kproj + vproj + nj=0 segment
            start_nj(0)
            for q in range(XQ):
                for kc in range(KC):
                    for nj in (2 * q, 2 * q + 1):
                        kproj(kc, nj)
                for mi in range(8 * q, 8 * q + 8):
                    vproj(mi)
                    loop_iter(0, mi)
            finish_nj(0)
            # remaining n-chunks
            for nj in range(1, NJ):
                start_nj(nj)
                for mi in range(MI):
                    loop_iter(nj, mi)
                finish_nj(nj)

        # ---- finalize: rowsums -> recip -> broadcast -> scale + bias -> out
        with tc.tile_pool(name="psF", bufs=2, space="PSUM") as psF, \
                tc.tile_pool(name="psT", bufs=1, space="PSUM") as psT, \
                tc.tile_pool(name="fin", bufs=6) as fin:
            for mi in range(MI):
                nc.vector.tensor_reduce(
                    rs_sb[:, mi:mi + 1],
                    rsparts_sb[:, mi * NJ:(mi + 1) * NJ],
                    axis=mybir.AxisListType.X, op=mybir.AluOpType.add)
            tp_ps = psT.tile([MI, P], f32, tag="tp", name="tp_ps")
            nc.tensor.transpose(tp_ps[:], rs_sb[:], ident[:])
            nc.vector.reciprocal(recipT_sb[:], tp_ps[:])
            # hi/lo bf16 split of the fp32 reciprocals (hi+lo reconstructs
            # fp32 to ~2^-16 rel); bf16 rank-1 matmuls rebuild fp32 in PSUM
            # at 1 cycle/row instead of walrus splitting an fp32 matmul in 2
            nc.vector.tensor_copy(recipT_hi[:], recipT_sb[:])
            nc.vector.tensor_sub(recipT_lo[:], recipT_sb[:], recipT_hi[:])
            nc.sync.dma_start(out=hirow_sb[:], in_=recipT_hi[:])
            nc.sync.dma_start(out=lorow_sb[:], in_=recipT_lo[:])

            for nj in range(NJ):
                bc_ps = psF.tile([P, NT], f32, tag="bc", name="bc_ps")
                nc.tensor.matmul(
                    bc_ps[:], lhsT=ones16[:],
                    rhs=hirow_sb[:, nj * NT:(nj + 1) * NT],
                    start=True, stop=False)
                nc.tensor.matmul(
                    bc_ps[:], lhsT=ones16[:],
                    rhs=lorow_sb[:, nj * NT:(nj + 1) * NT],
                    start=False, stop=True)
                for oc in range(OC):
                    ft = fin.tile([P, NT], f32, tag="ft", name="ft")
                    nc.vector.tensor_mul(
                        ft[:], outr_sb[oc][:, nj * NT:(nj + 1) * NT], bc_ps[:])
                    ot = fin.tile([P, NT], f32, tag="ot", name="ot")
                    nc.scalar.activation(ot[:], ft[:], AF.Identity,
                                         bias=bW_sb[:, oc:oc + 1])
                    nc.sync.dma_start(
                        out=out_d[oc * P:(oc + 1) * P, nj * NT:(nj + 1) * NT],
                        in_=ot[:])
    nc.compile()
    return nc


def _get_compiled():
    global _COMPILED
    if _COMPILED is None:
        _COMPILED = _build()
    return _COMPILED


def _make_in_maps(x, wv, bv, wk, bk, gamma, beta, rmean, rvar, wW, bW):
    x = np.asarray(x, dtype=np.float32)
    s = np.asarray(gamma, np.float32) / np.sqrt(np.asarray(rvar, np.float32) + BN_EPS)
    kscale = (s / 4.0).astype(np.float32).reshape(K, 1)
    kbias = (((np.asarray(bk, np.float32) - np.asarray(rmean, np.float32)) * s
              + np.asarray(beta, np.float32)) / 4.0).astype(np.float32).reshape(K, 1)
    shared = {
        "wkT16": np.ascontiguousarray(np.asarray(wk, np.float32).T).astype(BF16),
        "wvT16": np.ascontiguousarray(np.asarray(wv, np.float32).T).astype(BF16),
        "wWT16": np.ascontiguousarray(np.asarray(wW, np.float32).T).astype(BF16),
        "kscale": kscale,
        "kbias": kbias,
        "bvrow": np.asarray(bv, np.float32).reshape(1, V),
        "bW32": np.asarray(bW, np.float32).reshape(O, 1),
    }
    in_maps = []
    for b in range(B):
        m = dict(shared)
        m["x16"] = np.ascontiguousarray(x[b].reshape(C, N)).astype(BF16)
        in_maps.append(m)
    return in_maps


def _run(inputs, trace=False):
    from concourse.bass_utils import run_bass_kernel_spmd
    nc = _get_compiled()
    in_maps = _make_in_maps(**inputs)
    res = run_bass_kernel_spmd(nc, in_maps, list(range(B)), trace=trace)
    outs = [np.asarray(res.results[b]["out"], dtype=np.float32).reshape(O, H, W)
            for b in range(B)]
    return np.stack(outs), res


def kernel(x, wv, bv, wk, bk, gamma, beta, rmean, rvar, wW, bW):
    out, _ = _run(dict(x=x, wv=wv, bv=bv, wk=wk, bk=bk, gamma=gamma, beta=beta,
                       rmean=rmean, rvar=rvar, wW=wW, bW=bW))
    return out
